# revision 25
# baseline (speedup 1.0000x reference)
"""Trainium2 Bass kernel for nn_BinaryLinear: out = sign(x @ sign(W).T + bias).

Strategy
--------
Data-parallel over the 8192-token dim: each of the 8 cores gets 1024 tokens
and the full weight matrix.

On-chip compute (per core) is the NT GEMM z.T = sign(W) @ x.T on the
TensorEngine with the contraction (in_features) on the partition dim:

  psum[outf, tok] = sum_k w[k, outf] * x[k, tok]

Precision/speed: the moving operand x is kept in float32r ("fp32 reduced"):
the PE reads 4-byte fp32 and rounds to 11 explicit mantissa bits, RNE
(probed on HW with one-hot weights). HW-measured, an fp32r matmul with
free dim 512 issues at ~224ns — the same ~1 row/cycle rate as fp16/bf16/
fp8 (every matmul on this part streams one moving row per cycle at 2.4GHz
regardless of dtype; fp8 DoubleRow doubles contraction per instruction but
fp8's 3-bit mantissa then needs 3 planes = more instructions than one
12-bit fp32r pass). A single fp32r pass is the instruction-count optimum:

  32 k-tiles x 32 outf-tiles x 2 token-blocks = 2048 matmuls x ~224ns
  ~= 460us/core, vs 48 instr/block (~660us) for any fp16+fp8 / 3xfp8
  scheme. Weights +-1 are exact in any dtype; measured end-to-end
  rel_err 1.1e-2 vs the 2e-2 budget.

Layout/DMA: every transfer is contiguous per partition (strided-gather
DMAs on this part are descriptor-bound at ~4.6ns per element):
  - W ships as e4m3 +-1 pre-arranged host-side into per-block slabs
    w8[p, b, kt, m] so block b stages with ONE 512KB DMA (4KB/partition),
    then the otherwise-idle VectorE upconverts it to an fp32r SBUF tile
    (fp32r weights must go through the self-loading matmul; standalone
    ldweights is broken for 4-byte dtypes).
  - bias ships pre-transposed [128, MT] (the naive "(mo p) -> p mo"
    rearrange DMA is 4096 4-byte descriptors = 21.7us of queue time).
  - output is written as e4m3 (sign is +-1, exact) z.T [out_f, tok] and
    untransposed/upcast on the host.

Schedule: x k-tiles stream on two queues in token-halves (even k on
gpsimd, odd k interleaved with the first W slabs on sync; the n=0 matmuls
unblock on a half-arrived tile). The first four blocks run k-major
interleaved across all 8 PSUM banks — blocks 2,3 join at k>=JOIN_K and
sweep their deferred head afterwards — so the PE tracks the incoming x
stream instead of stalling on the last k-tile of block 0. Remaining
blocks run sequentially (2 banks each, 4-deep pipelined). The epilogue
(bias-add + sign + PSUM->SBUF in one ScalarE activation; bias is
per-partition in the z.T layout) plus output DMAs live on the scalar
queue so they never queue behind the W/x streams.

Measured: 495.4us HW exec (vs 710.2us for the previous fp16+fp8 hi/lo
kernel), rel_err 1.137e-2, 1085/33.5M sign flips — bit-identical to an
m11-RNE numpy simulation of the same inputs.
"""

import numpy as np

import concourse.tile as tile
import concourse.mybir as mybir
from concourse import bacc
from concourse.bass_utils import run_bass_kernel_spmd

N_CORES = 8
N_TOK = 8192
D_IN = 4096
D_OUT = 4096
P = 128
T = N_TOK // N_CORES  # 1024 tokens per core
KT = D_IN // P  # 32 contraction tiles
MT = D_OUT // P  # 32 out-feature tiles (= W blocks)
TB = 512  # token block (one PSUM bank of fp32)
NB = T // TB  # 2 token blocks per core
QUAD = 4  # leading blocks run k-major interleaved (8 PSUM banks)
JOIN_K = {0: 0, 1: 0, 2: 6, 3: 4}  # staggered joins (w32 casts land late)

F32 = mybir.dt.float32
F32R = mybir.dt.float32r
FP8 = mybir.dt.float8e4
SIGN = mybir.ActivationFunctionType.Sign
E4M3 = mybir.dt.np(FP8)

_nc_cache = None


def build():
    """Build + compile the per-core Bass/Tile module (SPMD: same on all cores)."""
    global _nc_cache
    if _nc_cache is not None:
        return _nc_cache
    nc = bacc.Bacc("TRN2", target_bir_lowering=False, debug=False, num_devices=N_CORES)
    x_d = nc.dram_tensor("x_t", [D_IN, T], F32R, kind="ExternalInput").ap()
    # per-block weight slabs: w8[p, b, kt, m] = sign(W)[b*128+m, kt*128+p]
    w_d = nc.dram_tensor("w8_slab", [P, MT, KT, P], FP8, kind="ExternalInput").ap()
    b_d = nc.dram_tensor("bias_t", [P, MT], F32, kind="ExternalInput").ap()
    out_d = nc.dram_tensor("out_t", [D_OUT, T], FP8, kind="ExternalOutput").ap()

    with tile.TileContext(nc) as tc:
        with (
            tc.tile_pool(name="x", bufs=1) as x_pool,
            tc.tile_pool(name="w8", bufs=3) as w8_pool,
            tc.tile_pool(name="w32", bufs=QUAD) as w32_pool,
            tc.tile_pool(name="bias", bufs=1) as b_pool,
            tc.tile_pool(name="out", bufs=6) as out_pool,
            tc.tile_pool(name="psum", bufs=8, space="PSUM") as psum_pool,
        ):
            xk = [
                x_pool.tile([P, T], F32R, tag=f"x_{k}", name=f"x_{k}")
                for k in range(KT)
            ]
            w8_tiles = {}
            w32_tiles = {}

            def stage_w8(b, queue=None):
                w8 = w8_pool.tile([P, KT, P], FP8, tag="w8", name=f"w8_{b}")
                (queue or nc.sync).dma_start(w8[:], w_d[:, b])
                w8_tiles[b] = w8

            COPY = mybir.ActivationFunctionType.Copy

            def cast_w(b, splits=2, engine="vector"):
                # k-split casts so early matmuls unblock after a partial slab;
                # the leading casts are spread over VectorE AND ScalarE so all
                # quad-phase blocks engage early.
                w32 = w32_pool.tile([P, KT, P], F32R, tag="w32", name=f"w32_{b}")
                w8 = w8_tiles.pop(b)
                h = KT // splits
                for s in range(splits):
                    src = w8[:, s * h : (s + 1) * h, :]
                    dst = w32[:, s * h : (s + 1) * h, :]
                    if engine == "vector":
                        nc.vector.tensor_copy(dst, src)
                    else:
                        nc.scalar.activation(dst, src, COPY)
                w32_tiles[b] = w32

            # x streams on two queues (even k on gpsimd, odd k on sync).
            # Transfers below ~4KB/partition are latency-bound (~2.4us for a
            # half OR a full tile), so only the first six k-tiles split into
            # token-halves (fine-grained unblocking for the quad phase); the
            # rest go as full 512KB tiles. W slabs 1,3 + bias + four late odd
            # tiles ride the scalar queue, which is idle until the epilogues.
            def dma_x(queue, k, halves=True):
                if halves:
                    for n in range(NB):
                        sl = slice(n * TB, (n + 1) * TB)
                        queue.dma_start(xk[k][:, sl], x_d[k * P : (k + 1) * P, sl])
                else:
                    queue.dma_start(xk[k][:], x_d[k * P : (k + 1) * P, :])

            stage_w8(0, nc.sync)
            stage_w8(1, nc.scalar)
            stage_w8(3, nc.scalar)
            dma_x(nc.sync, 1)
            for k in range(0, KT, 2):  # even k: gpsimd queue
                dma_x(nc.gpsimd, k)
            dma_x(nc.sync, 3)
            stage_w8(2, nc.sync)
            dma_x(nc.sync, 5)
            bias_sb = b_pool.tile([P, MT], F32, tag="bias")
            nc.scalar.dma_start(bias_sb[:], b_d[:, :])
            for k in range(7, KT, 2):  # remaining odd k on sync
                dma_x(nc.sync, k)

            cast_w(0, splits=4, engine="vector")
            cast_w(1, splits=2, engine="scalar")
            cast_w(2, splits=2, engine="vector")
            cast_w(3, splits=2, engine="scalar")

            nsls = [slice(n * TB, (n + 1) * TB) for n in range(NB)]

            def mm(psums, b, k, start, stop):
                for n in range(NB):
                    nc.tensor.matmul(
                        psums[(b, n)][:],
                        w32_tiles[b][:, k, :],
                        xk[k][:, nsls[n]],
                        start=start,
                        stop=stop,
                    )

            def epilogue(b, psums):
                for n in range(NB):
                    osb = out_pool.tile([P, TB], FP8, tag="osb",
                                        name=f"osb_{b}_{n}")
                    nc.scalar.activation(
                        osb[:], psums[(b, n)][:], SIGN,
                        bias=bias_sb[:, b : b + 1],
                    )
                    nc.scalar.dma_start(out_d[b * P : (b + 1) * P, nsls[n]], osb[:])

            # Quad phase: blocks 0..3 k-major across all 8 PSUM banks.
            qps = {
                (b, n): psum_pool.tile([P, TB], F32, tag="psum", name=f"ps_{b}_{n}")
                for b in range(QUAD)
                for n in range(NB)
            }
            def qmm(b, k, n, start, stop):
                nc.tensor.matmul(
                    qps[(b, n)][:],
                    w32_tiles[b][:, k, :],
                    xk[k][:, nsls[n]],
                    start=start,
                    stop=stop,
                )

            for k in range(KT):
                for n in range(NB):  # n-major: n=0 runs on half-arrived tiles
                    for b in range(QUAD):
                        if k >= JOIN_K[b]:
                            qmm(b, k, n, start=(k == JOIN_K[b]),
                                stop=(k == KT - 1 and JOIN_K[b] == 0))
            epilogue(0, qps)
            epilogue(1, qps)
            for b in (3, 2):  # sweep the deferred heads (b3 joined first)
                for n in range(NB):
                    for k in range(JOIN_K[b]):
                        qmm(b, k, n, start=False, stop=(k == JOIN_K[b] - 1))
                epilogue(b, qps)

            # Steady state: one block at a time, PSUM 4-deep pipelined.
            for b in range(QUAD, MT):
                stage_w8(b)
                cast_w(b)
                psums = {
                    (b, n): psum_pool.tile([P, TB], F32, tag="psum",
                                           name=f"ps_{b}_{n}")
                    for n in range(NB)
                }
                for k in range(KT):
                    mm(psums, b, k, start=(k == 0), stop=(k == KT - 1))
                epilogue(b, psums)
    nc.compile()
    _nc_cache = nc
    return nc


def prep_in_maps(x, weight, bias):
    """Host-side layout prep: transposes, fp8 sign-weight slabs, token shards."""
    x = np.asarray(x, dtype=np.float32)
    weight = np.asarray(weight, dtype=np.float32)
    bias = np.asarray(bias, dtype=np.float32)

    x_t = np.ascontiguousarray(x.T)  # [D_IN, N_TOK]
    # w8_slab[p, b, kt, m] = sign(W)[b*128+m, kt*128+p]
    w8 = np.sign(weight).astype(E4M3).reshape(MT, P, KT, P)
    w8_slab = np.ascontiguousarray(w8.transpose(3, 0, 2, 1))
    bias_t = np.ascontiguousarray(bias.reshape(MT, P).T)  # [P, MT]

    in_maps = []
    for c in range(N_CORES):
        sl = slice(c * T, (c + 1) * T)
        in_maps.append(
            {
                "x_t": np.ascontiguousarray(x_t[:, sl]),
                "w8_slab": w8_slab,
                "bias_t": bias_t,
            }
        )
    return in_maps


def run(x, weight, bias, **spmd_kwargs):
    """Run on the 8 cores; returns (full_output, BassKernelResults)."""
    nc = build()
    in_maps = prep_in_maps(x, weight, bias)
    res = run_bass_kernel_spmd(nc, in_maps, core_ids=list(range(N_CORES)), **spmd_kwargs)
    out = np.empty((N_TOK, D_OUT), dtype=np.float32)
    for c in range(N_CORES):
        out[c * T : (c + 1) * T, :] = res.results[c]["out_t"].astype(np.float32).T
    return out, res


def kernel(x, weight, bias):
    out, _ = run(x, weight, bias)
    return out


# revision 28
# speedup vs baseline: 1.0071x; 1.0071x over previous
"""Trainium2 Bass kernel for nn_BinaryLinear: out = sign(x @ sign(W).T + bias).

Strategy
--------
Data-parallel over the 8192-token dim: each of the 8 cores gets 1024 tokens
and the full weight matrix.

On-chip compute (per core) is the NT GEMM z.T = sign(W) @ x.T on the
TensorEngine with the contraction (in_features) on the partition dim:

  psum[outf, tok] = sum_k w[k, outf] * x[k, tok]

Precision/speed: the moving operand x is kept in float32r ("fp32 reduced"):
the PE reads 4-byte fp32 and rounds to 11 explicit mantissa bits, RNE
(probed on HW with one-hot weights). HW-measured, an fp32r matmul with
free dim 512 issues at ~224ns — the same ~1 row/cycle rate as fp16/bf16/
fp8 (every matmul on this part streams one moving row per cycle at 2.4GHz
regardless of dtype; fp8 DoubleRow doubles contraction per instruction but
fp8's 3-bit mantissa then needs 3 planes = more instructions than one
12-bit fp32r pass). A single fp32r pass is the instruction-count optimum:

  32 k-tiles x 32 outf-tiles x 2 token-blocks = 2048 matmuls x ~224ns
  ~= 460us/core, vs 48 instr/block (~660us) for any fp16+fp8 / 3xfp8
  scheme. Weights +-1 are exact in any dtype; measured end-to-end
  rel_err 1.1e-2 vs the 2e-2 budget.

Layout/DMA: every transfer is contiguous per partition (strided-gather
DMAs on this part are descriptor-bound at ~4.6ns per element):
  - W ships as e4m3 +-1 pre-arranged host-side into per-block slabs
    w8[p, b, kt, m] so block b stages with ONE 512KB DMA (4KB/partition),
    then the otherwise-idle VectorE upconverts it to an fp32r SBUF tile
    (fp32r weights must go through the self-loading matmul; standalone
    ldweights is broken for 4-byte dtypes).
  - bias ships pre-transposed [128, MT] (the naive "(mo p) -> p mo"
    rearrange DMA is 4096 4-byte descriptors = 21.7us of queue time).
  - output is written as e4m3 (sign is +-1, exact) z.T [out_f, tok] and
    untransposed/upcast on the host.

Schedule: x k-tiles stream on two queues in token-halves (even k on
gpsimd, odd k interleaved with the first W slabs on sync; the n=0 matmuls
unblock on a half-arrived tile). The first four blocks run k-major
interleaved across all 8 PSUM banks — blocks 2,3 join at k>=JOIN_K and
sweep their deferred head afterwards — so the PE tracks the incoming x
stream instead of stalling on the last k-tile of block 0. Remaining
blocks run sequentially (2 banks each, 4-deep pipelined). The epilogue
(bias-add + sign + PSUM->SBUF in one ScalarE activation; bias is
per-partition in the z.T layout) plus output DMAs live on the scalar
queue so they never queue behind the W/x streams.

Measured: 495.4us HW exec (vs 710.2us for the previous fp16+fp8 hi/lo
kernel), rel_err 1.137e-2, 1085/33.5M sign flips — bit-identical to an
m11-RNE numpy simulation of the same inputs.
"""

import numpy as np

import concourse.tile as tile
import concourse.mybir as mybir
from concourse import bacc
from concourse.bass_utils import run_bass_kernel_spmd

N_CORES = 8
N_TOK = 8192
D_IN = 4096
D_OUT = 4096
P = 128
T = N_TOK // N_CORES  # 1024 tokens per core
KT = D_IN // P  # 32 contraction tiles
MT = D_OUT // P  # 32 out-feature tiles (= W blocks)
TB = 512  # token block (one PSUM bank of fp32)
NB = T // TB  # 2 token blocks per core
QUAD = 4  # leading blocks run k-major interleaved (8 PSUM banks)
JOIN_K = {0: 0, 1: 0, 2: 6, 3: 4}  # staggered joins (w32 casts land late)

F32 = mybir.dt.float32
F32R = mybir.dt.float32r
FP8 = mybir.dt.float8e4
SIGN = mybir.ActivationFunctionType.Sign
E4M3 = mybir.dt.np(FP8)

_nc_cache = None


def build():
    """Build + compile the per-core Bass/Tile module (SPMD: same on all cores)."""
    global _nc_cache
    if _nc_cache is not None:
        return _nc_cache
    nc = bacc.Bacc("TRN2", target_bir_lowering=False, debug=False, num_devices=N_CORES)
    x_d = nc.dram_tensor("x_t", [D_IN, T], F32R, kind="ExternalInput").ap()
    # per-block weight slabs: w8[p, b, kt, m] = sign(W)[b*128+m, kt*128+p]
    w_d = nc.dram_tensor("w8_slab", [P, MT, KT, P], FP8, kind="ExternalInput").ap()
    b_d = nc.dram_tensor("bias_t", [P, MT], F32, kind="ExternalInput").ap()
    out_d = nc.dram_tensor("out_t", [D_OUT, T], FP8, kind="ExternalOutput").ap()

    with tile.TileContext(nc) as tc:
        with (
            tc.tile_pool(name="x", bufs=1) as x_pool,
            tc.tile_pool(name="w8", bufs=3) as w8_pool,
            tc.tile_pool(name="w32", bufs=QUAD) as w32_pool,
            tc.tile_pool(name="bias", bufs=1) as b_pool,
            tc.tile_pool(name="out", bufs=6) as out_pool,
            tc.tile_pool(name="psum", bufs=8, space="PSUM") as psum_pool,
        ):
            xk = [
                x_pool.tile([P, T], F32R, tag=f"x_{k}", name=f"x_{k}")
                for k in range(KT)
            ]
            w8_tiles = {}
            w32_tiles = {}

            def stage_w8(b, queue=None, split=False):
                w8 = w8_pool.tile([P, KT, P], FP8, tag="w8", name=f"w8_{b}")
                q = queue or nc.sync
                if split:  # first k-quarter lands early -> earlier MM#0
                    q.dma_start(w8[:, : KT // 4, :], w_d[:, b, : KT // 4])
                    q.dma_start(w8[:, KT // 4 :, :], w_d[:, b, KT // 4 :])
                else:
                    q.dma_start(w8[:], w_d[:, b])
                w8_tiles[b] = w8

            COPY = mybir.ActivationFunctionType.Copy

            def cast_w(b, splits=2, engine="vector"):
                # k-split casts so early matmuls unblock after a partial slab;
                # the leading casts are spread over VectorE AND ScalarE so all
                # quad-phase blocks engage early.
                w32 = w32_pool.tile([P, KT, P], F32R, tag="w32", name=f"w32_{b}")
                w8 = w8_tiles.pop(b)
                h = KT // splits
                for s in range(splits):
                    src = w8[:, s * h : (s + 1) * h, :]
                    dst = w32[:, s * h : (s + 1) * h, :]
                    if engine == "vector":
                        nc.vector.tensor_copy(dst, src)
                    else:
                        nc.scalar.activation(dst, src, COPY)
                w32_tiles[b] = w32

            # x streams on two queues (even k on gpsimd, odd k on sync).
            # Transfers below ~4KB/partition are latency-bound (~2.4us for a
            # half OR a full tile), so only the first six k-tiles split into
            # token-halves (fine-grained unblocking for the quad phase); the
            # rest go as full 512KB tiles. W slabs 1,3 + bias + four late odd
            # tiles ride the scalar queue, which is idle until the epilogues.
            def dma_x(queue, k, halves=True):
                if halves:
                    for n in range(NB):
                        sl = slice(n * TB, (n + 1) * TB)
                        queue.dma_start(xk[k][:, sl], x_d[k * P : (k + 1) * P, sl])
                else:
                    queue.dma_start(xk[k][:], x_d[k * P : (k + 1) * P, :])

            stage_w8(0, nc.sync, split=True)
            stage_w8(1, nc.scalar)
            stage_w8(3, nc.scalar)
            dma_x(nc.sync, 1)
            for k in range(0, KT, 2):  # even k: gpsimd queue
                dma_x(nc.gpsimd, k)
            dma_x(nc.sync, 3)
            stage_w8(2, nc.sync)
            dma_x(nc.sync, 5)
            bias_sb = b_pool.tile([P, MT], F32, tag="bias")
            nc.scalar.dma_start(bias_sb[:], b_d[:, :])
            for k in range(7, KT, 2):  # remaining odd k on sync
                dma_x(nc.sync, k)

            cast_w(0, splits=4, engine="vector")
            cast_w(1, splits=2, engine="scalar")
            cast_w(2, splits=2, engine="vector")
            cast_w(3, splits=2, engine="scalar")

            nsls = [slice(n * TB, (n + 1) * TB) for n in range(NB)]

            def mm(psums, b, k, start, stop):
                for n in range(NB):
                    nc.tensor.matmul(
                        psums[(b, n)][:],
                        w32_tiles[b][:, k, :],
                        xk[k][:, nsls[n]],
                        start=start,
                        stop=stop,
                    )

            def epilogue(b, psums):
                for n in range(NB):
                    osb = out_pool.tile([P, TB], FP8, tag="osb",
                                        name=f"osb_{b}_{n}")
                    nc.scalar.activation(
                        osb[:], psums[(b, n)][:], SIGN,
                        bias=bias_sb[:, b : b + 1],
                    )
                    nc.scalar.dma_start(out_d[b * P : (b + 1) * P, nsls[n]], osb[:])

            # Quad phase: blocks 0..3 k-major across all 8 PSUM banks.
            qps = {
                (b, n): psum_pool.tile([P, TB], F32, tag="psum", name=f"ps_{b}_{n}")
                for b in range(QUAD)
                for n in range(NB)
            }
            def qmm(b, k, n, start, stop):
                nc.tensor.matmul(
                    qps[(b, n)][:],
                    w32_tiles[b][:, k, :],
                    xk[k][:, nsls[n]],
                    start=start,
                    stop=stop,
                )

            # Iterate k in the tiles' actual ARRIVAL order (even k on gpsimd
            # runs ~one tile ahead of odd k on sync, which carries the W
            # slabs first); PSUM accumulation order within a group is free.
            qorder = [0]
            for e, o in zip(range(2, KT, 2), range(1, KT, 2)):
                qorder += [e, o]
            qorder.append(KT - 1)

            for i, k in enumerate(qorder):
                for n in range(NB):  # n-major: n=0 runs on half-arrived tiles
                    for b in range(QUAD):
                        if i >= JOIN_K[b]:
                            qmm(b, k, n, start=(i == JOIN_K[b]),
                                stop=(i == KT - 1 and JOIN_K[b] == 0))
            epilogue(0, qps)
            epilogue(1, qps)
            for b in (3, 2):  # sweep the deferred heads (b3 joined first)
                head = qorder[: JOIN_K[b]]
                for n in range(NB):
                    for j, k in enumerate(head):
                        qmm(b, k, n, start=False, stop=(j == len(head) - 1))
                epilogue(b, qps)

            # Steady state: one block at a time, PSUM 4-deep pipelined.
            for b in range(QUAD, MT):
                stage_w8(b)
                cast_w(b)
                psums = {
                    (b, n): psum_pool.tile([P, TB], F32, tag="psum",
                                           name=f"ps_{b}_{n}")
                    for n in range(NB)
                }
                for k in range(KT):
                    mm(psums, b, k, start=(k == 0), stop=(k == KT - 1))
                epilogue(b, psums)
    nc.compile()
    _nc_cache = nc
    return nc


def prep_in_maps(x, weight, bias):
    """Host-side layout prep: transposes, fp8 sign-weight slabs, token shards."""
    x = np.asarray(x, dtype=np.float32)
    weight = np.asarray(weight, dtype=np.float32)
    bias = np.asarray(bias, dtype=np.float32)

    x_t = np.ascontiguousarray(x.T)  # [D_IN, N_TOK]
    # w8_slab[p, b, kt, m] = sign(W)[b*128+m, kt*128+p]
    w8 = np.sign(weight).astype(E4M3).reshape(MT, P, KT, P)
    w8_slab = np.ascontiguousarray(w8.transpose(3, 0, 2, 1))
    bias_t = np.ascontiguousarray(bias.reshape(MT, P).T)  # [P, MT]

    in_maps = []
    for c in range(N_CORES):
        sl = slice(c * T, (c + 1) * T)
        in_maps.append(
            {
                "x_t": np.ascontiguousarray(x_t[:, sl]),
                "w8_slab": w8_slab,
                "bias_t": bias_t,
            }
        )
    return in_maps


def run(x, weight, bias, **spmd_kwargs):
    """Run on the 8 cores; returns (full_output, BassKernelResults)."""
    nc = build()
    in_maps = prep_in_maps(x, weight, bias)
    res = run_bass_kernel_spmd(nc, in_maps, core_ids=list(range(N_CORES)), **spmd_kwargs)
    out = np.empty((N_TOK, D_OUT), dtype=np.float32)
    for c in range(N_CORES):
        out[c * T : (c + 1) * T, :] = res.results[c]["out_t"].astype(np.float32).T
    return out, res


def kernel(x, weight, bias):
    out, _ = run(x, weight, bias)
    return out
